# revision 15
# baseline (speedup 1.0000x reference)
"""Sharded attention kernel v2 for Trainium2 (8 NeuronCores).

softmax(q @ k^T / sqrt(d) + mask) @ v ; q,k,v [8192,128] f32, mask [8192,8192].

Sharding: q/mask rows split 8 ways (1024 rows/core); k, v replicated.

Key changes vs v1 (115 us):
 - exp moved to ACT reading raw f32 scores straight from PSUM (64 big
   [128,1024] activations instead of 128 small fp16 ones); q pre-scaled by
   1/sqrt(d) on host so the activation is a pure Exp.
 - mask applied AFTER exp as a multiply by host-precomputed exp(mask)
   (softmax(s+m) == exp(s)*exp(m) / sum), on DVE at 2x (bf16 mask) or
   DVE/GPSIMD split (fp8 mask).
 - PE stream made gap-free: warm-up dummy matmuls ramp the tensor engine to
   its full 2.4 GHz p-state before real work, and a tunable per-group pad
   matmul keeps PE the (slightly) slowest engine so it never idles/resets.
 - loop order: h (q-half) outer, 32 "groups" of 2 key-blocks inner; PSUM
   plan: scores 3x[128,2,512]f32 (6 banks) + out accum 2x[128,258]f32
   (2 banks) = 8 banks; exp output lives in SBUF (fp16 PSUM writes are
   not valid ISA for the Activation engine).
"""

import numpy as np

import concourse.bacc as bacc
import concourse.mybir as mybir
import concourse.tile as tile
from concourse.bass import ds, ts
from concourse.bass_utils import run_bass_kernel_spmd

N = 8192
M = 8192
D = 128
P = 128
NCORES = 8
N_SH = N // NCORES  # 1024 q rows per core
HW = 512  # q-half width
NB = 64  # key blocks of 128
NGP = 32  # groups per half (2 key blocks each)
NGRP = 2 * NGP  # 64 groups total
GW = 1024  # group score width per partition (2 blocks x 512 n)
SCALE = 1.0 / float(np.sqrt(D))

F32 = mybir.dt.float32
F16 = mybir.dt.float16
BF16 = mybir.dt.bfloat16
FP8 = mybir.dt.float8e4
MULT = mybir.AluOpType.mult
EXP = mybir.ActivationFunctionType.Exp

# --- tunables ---
EM_MODE = "bf16"  # "bf16": all-DVE mask mul; "fp8": DVE/GPSIMD split
PAD = 280  # per-group PE pad rows (0 = off)
NDUM = 6  # warm-up dummy matmuls ([128,512] each)
NBDUM = 4  # h-boundary dummy matmuls
PF = 12  # em prefetch distance (groups)
GPS_EVERY = 5  # fp8 mode: every GPS_EVERY-th group's mul goes to gpsimd


def build_nc(em_mode=None, pad=None, ndum=None, nbdum=None):
    em_mode = EM_MODE if em_mode is None else em_mode
    pad = PAD if pad is None else pad
    ndum = NDUM if ndum is None else ndum
    nbdum = NBDUM if nbdum is None else nbdum
    em_dt = BF16 if em_mode == "bf16" else FP8

    nc = bacc.Bacc(None, target_bir_lowering=False)
    qt = nc.dram_tensor("qt", [D, N_SH], F16, kind="ExternalInput")
    kt = nc.dram_tensor("kt", [D, M], F16, kind="ExternalInput")
    vaug_d = nc.dram_tensor("vaug", [P, NB, D + 1], F16, kind="ExternalInput")
    em_d = nc.dram_tensor("em", [2, NGP, P, GW], em_dt, kind="ExternalInput")
    out = nc.dram_tensor("out", [N_SH, D], F32, kind="ExternalOutput")

    with tile.TileContext(nc) as tc:
        with (
            tc.tile_pool(name="big", bufs=1) as big_pool,
            tc.tile_pool(name="emp", bufs=14) as em_pool,
            tc.tile_pool(name="ptp", bufs=6) as pt_pool,
            tc.tile_pool(name="op", bufs=4) as o_pool,
            tc.tile_pool(name="ps_s", bufs=3, space="PSUM") as ps_s_pool,
            tc.tile_pool(name="pexp", bufs=4) as pexp_pool,
            tc.tile_pool(name="ps_o", bufs=2, space="PSUM") as ps_o_pool,
        ):
            qt_all = big_pool.tile([P, N_SH], F16)
            kt_t = big_pool.tile([P, M], F16)
            vaug = big_pool.tile([P, NB, D + 1], F16)
            scr = big_pool.tile([P, HW], F16)

            # -- PE warm-up source + ACT table priming first: the Pool
            # queue serializes DMA transfers, so the memset must precede the
            # em stream or the warm-up dummies stall ~10us --
            nc.gpsimd.memset(scr[:], 0.0)
            prime = o_pool.tile([P, 8], F16, tag="prime")
            nc.scalar.activation(prime[:], scr[:, 0:8], EXP)

            # -- initial DMAs, earliest-needed first --
            em_tiles = {}

            def _em_load(g):
                # First tiles gate pipeline fill: halve them so two DMA
                # engines carry each in parallel (~2.8us latency vs ~5.7).
                # Only the first few -- more pieces cuts the Pool queue's
                # issue rate below the pipeline pace, and pieces on the SP
                # queue would sit behind the kt/vaug init transfers.
                t = em_pool.tile([P, GW], em_dt, tag="em", name=f"em{g}")
                pieces = 1
                w = GW // pieces
                for i in range(pieces):
                    nc.gpsimd.dma_start(
                        t[:, ds(i * w, w)],
                        em_d[g // NGP, g % NGP, :, ds(i * w, w)],
                    )
                em_tiles[g] = t

            nc.sync.dma_start(qt_all[:], qt[:])
            nc.sync.dma_start(kt_t[:, ds(0, 512)], kt[:, ds(0, 512)])
            _em_load(0)
            nc.sync.dma_start(kt_t[:, ds(512, 1536)], kt[:, ds(512, 1536)])
            _em_load(1)
            nc.sync.dma_start(vaug[:, 0:16, :], vaug_d[:, 0:16, :])
            for g in range(2, PF):
                _em_load(g)
            for i in range(1, 4):
                nc.sync.dma_start(
                    kt_t[:, ds(i * 2048, 2048)], kt[:, ds(i * 2048, 2048)]
                )
            for i in range(1, 4):
                nc.sync.dma_start(
                    vaug[:, ds(i * 16, 16), :], vaug_d[:, ds(i * 16, 16), :]
                )

            st = {}
            # first ps_s tile doubles as the dummy-matmul target
            st["s", 0] = ps_s_pool.tile([P, 2, HW], F32, tag="ps_s", name="ps_s0")
            for _ in range(ndum):
                nc.tensor.matmul(
                    st["s", 0][:, 0, :],
                    scr[:, 0:P],
                    scr[:],
                    start=True,
                    stop=True,
                )

            def stage_m(g):
                # scores for group g: 2 key blocks x current q-half
                h, gp = divmod(g, NGP)
                if ("s", g) not in st:
                    st["s", g] = ps_s_pool.tile(
                        [P, 2, HW], F32, tag="ps_s", name=f"ps_s{g}"
                    )
                ps_s = st["s", g]
                if pad and g > 0:
                    nc.tensor.matmul(
                        ps_s[:, 0, ds(0, pad)],
                        scr[:, 0:P],
                        scr[:, ds(0, pad)],
                        start=True,
                        stop=True,
                    )
                for j in range(2):
                    nc.tensor.matmul(
                        ps_s[:, j, :],
                        kt_t[:, ts(2 * gp + j, P)],
                        qt_all[:, ds(h * HW, HW)],
                        start=True,
                        stop=True,
                    )

            def stage_dma(g):
                _em_load(g)

            def stage_e(g):
                ps_s = st.pop(("s", g))
                pe = pexp_pool.tile([P, GW], F16, tag="pexp", name=f"pexp{g}")
                nc.scalar.activation(pe[:], ps_s[:], EXP)
                st["e", g] = pe

            def stage_t(g):
                pe = st.pop(("e", g))
                emt = em_tiles.pop(g)
                p_t = pt_pool.tile([P, GW], F16, tag="pt", name=f"pt{g}")
                eng = (
                    nc.gpsimd
                    if (em_mode == "fp8" and g % GPS_EVERY == GPS_EVERY - 1)
                    else nc.vector
                )
                eng.tensor_tensor(p_t[:], pe[:], emt[:], op=MULT)
                st["p", g] = p_t

            def stage_v(g):
                h, gp = divmod(g, NGP)
                p_t = st.pop(("p", g))
                if gp == 0:
                    for half in range(2):
                        st["o", h, half] = ps_o_pool.tile(
                            [P, 2 * (D + 1)],
                            F32,
                            tag="ps_o",
                            name=f"ps_o{h}_{half}",
                        )
                # One PSUM accumulation group per bank: start claims (and
                # lazily zeroes) the whole 2KB bank, so only the first
                # slice's b==0 matmul starts and only the last slice's
                # b==NB-1 matmul stops.
                for j in range(2):
                    b = 2 * gp + j
                    for t in range(4):
                        nc.tensor.matmul(
                            st["o", h, t // 2][:, ds((t % 2) * (D + 1), D + 1)],
                            p_t[:, ds(j * HW + t * P, P)],
                            vaug[:, b, :],
                            start=(b == 0 and t % 2 == 0),
                            stop=(b == NB - 1 and t % 2 == 1),
                        )

            def stage_norm(h, t):
                ps_o = st["o", h, t // 2][:, ds((t % 2) * (D + 1), D + 1)]
                l_r = o_pool.tile([P, 1], F32, tag="lr", name=f"lr{h}_{t}")
                nc.vector.reciprocal(l_r[:], ps_o[:, D : D + 1])
                o_sb = o_pool.tile([P, D], F32, tag="osb", name=f"osb{h}_{t}")
                nc.vector.tensor_scalar(
                    o_sb[:], ps_o[:, 0:D], l_r[:], None, op0=MULT
                )
                nc.sync.dma_start(out[ts(h * 4 + t, P), :], o_sb[:])

            # -- main pipeline --
            # per step i: mm1(i+2) [PE], em-dma(i+PF), exp(i+1) [ACT],
            #             mul(i+1) [DVE/Pool], mm2(i) [PE]
            stage_m(0)
            stage_m(1)
            stage_e(0)
            stage_t(0)
            for i in range(NGRP):
                if i + 2 < NGRP:
                    stage_m(i + 2)
                if i + PF < NGRP:
                    stage_dma(i + PF)
                if i + 1 < NGRP:
                    stage_e(i + 1)
                    stage_t(i + 1)
                stage_v(i)
                if i == NGP - 1:
                    # h boundary: keep PE busy while h=0 norms drain ps_o.
                    # Dummies write into the tile stage_m(i+3) will use; its
                    # buffer is ps_s(i+1)'s, so the WAR wait on exp(i+1) is
                    # exactly the stall window we're covering, and stage_m
                    # fully overwrites the garbage.
                    st["s", i + 3] = ps_s_pool.tile(
                        [P, 2, HW], F32, tag="ps_s", name=f"ps_s{i + 3}"
                    )
                    for _ in range(nbdum):
                        nc.tensor.matmul(
                            st["s", i + 3][:, 0, :],
                            scr[:, 0:P],
                            scr[:],
                            start=True,
                            stop=True,
                        )
                    for t in range(4):
                        stage_norm(0, t)
                    for half in range(2):
                        st.pop(("o", 0, half))
            for t in range(4):
                stage_norm(1, t)
            for half in range(2):
                st.pop(("o", 1, half))

    nc.compile()
    return nc


_CACHE = {}


def _get_nc():
    if "nc" not in _CACHE:
        _CACHE["nc"] = build_nc()
    return _CACHE["nc"]


def make_host_tensors(q, k, v, mask, em_mode=None):
    """Shared host marshalling: returns (qt per-core list, kt, vaug, em list)."""
    import ml_dtypes

    em_mode = EM_MODE if em_mode is None else em_mode
    em_np = ml_dtypes.bfloat16 if em_mode == "bf16" else ml_dtypes.float8_e4m3

    q = (np.asarray(q, dtype=np.float32) * SCALE).astype(np.float16)
    kt = np.ascontiguousarray(np.asarray(k).astype(np.float16).T)  # [D, M]
    v16 = np.asarray(v).astype(np.float16)
    vaug = np.ones((P, NB, D + 1), dtype=np.float16)
    vaug[:, :, 0:D] = v16.reshape(NB, P, D).transpose(1, 0, 2)
    vaug = np.ascontiguousarray(vaug)

    mask = np.asarray(mask, dtype=np.float32)
    in_maps = []
    for c in range(NCORES):
        sl = slice(c * N_SH, (c + 1) * N_SH)
        if not mask[sl].any():
            em = np.ones((2, NGP, P, GW), dtype=em_np)
        else:
            em_f = np.exp(mask[sl]).T  # [M, N_SH]
            em_f = em_f.reshape(NB, P, 2, HW)  # [b, p, h, j]
            em_f = em_f.transpose(2, 0, 1, 3)  # [h, b, p, j]
            em_f = em_f.reshape(2, NGP, 2, P, HW).transpose(0, 1, 3, 2, 4)
            em = np.ascontiguousarray(
                em_f.reshape(2, NGP, P, GW).astype(em_np)
            )
        in_maps.append(
            {
                "qt": np.ascontiguousarray(q[sl].T),
                "kt": kt,
                "vaug": vaug,
                "em": em,
            }
        )
    return in_maps


def _run(q, k, v, mask, **spmd_kwargs):
    nc = _get_nc()
    res = run_bass_kernel_spmd(
        nc,
        make_host_tensors(q, k, v, mask),
        core_ids=list(range(NCORES)),
        **spmd_kwargs,
    )
    full = np.concatenate(
        [res.results[c]["out"] for c in range(NCORES)], axis=0
    ).astype(np.float32)
    return full, res


def kernel(q, k, v, mask):
    full, _ = _run(q, k, v, mask)
    return full
